# revision 1
# baseline (speedup 1.0000x reference)
"""VQ-codebook 3x3 conv (nn_CConv) on 8 Trainium2 NeuronCores.

Sharding: data-parallel over the batch (16 images -> 2 per core); the small
codebook-derived weights / scales / bias are replicated to every core.
Host-side work is layout only: batch split, reshape/transpose of the index
and scale matrices, and the codebook row gather (pure indexing, no
arithmetic).

Per-core device program (one NEFF, SPMD over 8 cores):
  - weight build (on device): fp16 round-trip of scales (dequant emulation),
    multiply by cut, broadcast-multiply onto the gathered codebook rows;
    weights stored k-major so each of the 9 taps is a contiguous
    [128(in), 128(out)] fp16 stationary block.
  - conv: each image is zero-padded to rows of width 114 in SBUF (borders
    zeroed on-chip); the 3x3 conv is 9 accumulating PE matmuls over shifted
    views of the flattened padded image, fp16 in / fp32 PSUM accumulate.
    Outputs are computed in "padded q space" (q = h*114 + w); junk columns
    w in {112,113} are computed but dropped by the strided output DMA.
  - images are processed in row-slabs (8/20/28 output rows; tiny slabs at the
    kernel's two ends shorten the serial prologue and the final-DMA tail);
    input loads (sync HWDGE queue) are double-buffered and cast f32->f16 by
    the scalar engine; PSUM is evacuated with a fused per-partition bias add
    on the vector engine; output DMAs ride the scalar HWDGE queue so they
    never head-of-line-block input loads.
  - 14 dummy warm-up matmuls run during the prologue so the PE HAM clock
    gate reaches 2.4 GHz before the real matmuls start.
"""
import sys
import types
from contextlib import ExitStack

import numpy as np

import concourse.tile as tile
from concourse import bacc, mybir


def _ensure_axon_hooks_module():
    """This image's antenv package lacks axon_hooks; bass_utils imports it
    when tracing is requested (e.g. BASS_TRACE=1). Provide a no-op shim."""
    try:
        import antenv

        if "antenv.axon_hooks" not in sys.modules and not hasattr(
            antenv, "axon_hooks"
        ):
            mod = types.ModuleType("antenv.axon_hooks")
            holder = [None]
            mod.set_axon_ntff_profile_hook = lambda h: holder.__setitem__(0, h)
            mod.get_axon_ntff_profile_hook = lambda: holder[0]
            antenv.axon_hooks = mod
            sys.modules["antenv.axon_hooks"] = mod
    except Exception:
        pass


_ensure_axon_hooks_module()

from concourse import bass_utils  # noqa: E402

P = 128
H = W = 112
WP = 114
IMGS = 2
N_CORES = 8

f32 = mybir.dt.float32
f16 = mybir.dt.float16

SLAB_PLAN = {0: [8, 20, 28, 28, 28], 1: [28, 28, 28, 20, 8]}
MAX_SO = 28
WARM_MMS = 14

_CACHE = {}


def _slab_tiles(slab_out):
    n_pos = slab_out * WP - 2
    full, r = divmod(n_pos, 512)
    tiles = [512] * full
    if r:
        if r < 256 and full:
            tiles = [512] * (full - 1) + [(512 + r) // 2, (512 + r) - (512 + r) // 2]
        else:
            tiles.append(r)
    assert sum(tiles) == n_pos
    return tiles


def _build():
    nc = bacc.Bacc("TRN2", target_bir_lowering=False, debug=False)

    x_t = nc.dram_tensor("x", [IMGS, P, H, W], f32, kind="ExternalInput")
    scalesT_t = nc.dram_tensor("scalesT", [P, P], f32, kind="ExternalInput")
    cutT_t = nc.dram_tensor("cutT", [P, P], f32, kind="ExternalInput")
    bias_t = nc.dram_tensor("bias", [P, 1], f32, kind="ExternalInput")
    wrawT_t = nc.dram_tensor("wrawT", [P, P * 9], f32, kind="ExternalInput")
    out_t = nc.dram_tensor("out", [IMGS, P, H, W], f32, kind="ExternalOutput")

    with tile.TileContext(nc) as tc, ExitStack() as ctx:
        wb = ctx.enter_context(tc.tile_pool(name="wb", bufs=1))
        xp = ctx.enter_context(tc.tile_pool(name="xp", bufs=4))
        op = ctx.enter_context(tc.tile_pool(name="op", bufs=4))
        ps = ctx.enter_context(tc.tile_pool(name="ps", bufs=6, space="PSUM"))
        xs = ctx.enter_context(tc.tile_pool(name="xs", bufs=4))

        # PE warmup: HAM un-throttles to 2.4 GHz during the prologue
        wrm = wb.tile([P, 512], f16, tag="warm")
        nc.gpsimd.memset(wrm[:], 0.0)
        pw = ps.tile([P, 512], f32, tag="pst")
        for _ in range(WARM_MMS):
            nc.tensor.matmul(pw[:], wrm[:, :P], wrm[:], start=True, stop=True)

        # peel slab (0,0) input load so it heads the sync DMA queue
        so0 = SLAB_PLAN[0][0]
        nrows0 = min(H, so0 + 1)
        pre_stage = xs.tile([P, (MAX_SO + 2) * W], f32, tag="xstage")
        nc.sync.dma_start(pre_stage[:, :nrows0 * W], x_t.ap()[0, :, 0:nrows0, :])

        # ---- weight build ----
        w_raw = wb.tile([P, P * 9], f32, tag="w_raw")
        nc.sync.dma_start(w_raw[:], wrawT_t.ap())
        sc_in = wb.tile([P, P], f32, tag="sc_in")
        nc.sync.dma_start(sc_in[:], scalesT_t.ap())
        cut_s = wb.tile([P, P], f32, tag="cut")
        nc.sync.dma_start(cut_s[:], cutT_t.ap())
        bias_s = wb.tile([P, 1], f32, tag="bias")
        nc.sync.dma_start(bias_s[:], bias_t.ap())

        sc16 = wb.tile([P, P], f16, tag="sc16")
        nc.vector.tensor_copy(sc16[:], sc_in[:])
        sc = wb.tile([P, P], f32, tag="sc")
        nc.vector.tensor_copy(sc[:], sc16[:])
        scc = wb.tile([P, P], f32, tag="scc")
        nc.vector.tensor_tensor(
            out=scc[:], in0=sc[:], in1=cut_s[:], op=mybir.AluOpType.mult
        )

        # w_mm[i, k, o] = w_raw[i, o, k] * scc[i, o]
        w_mm = wb.tile([P, 9 * P], f16, tag="w_mm")
        w_raw3 = w_raw[:].rearrange("p (o k) -> p k o", k=9)
        scc3 = scc[:].rearrange("p (one o) -> p one o", one=1).to_broadcast(
            [P, 9, P]
        )
        w_mm3 = w_mm[:].rearrange("p (k o) -> p k o", o=P)
        nc.vector.tensor_tensor(
            out=w_mm3, in0=w_raw3, in1=scc3, op=mybir.AluOpType.mult
        )
        w_k_view = w_mm[:].rearrange("p (k o) -> p k o", o=P)

        # ---- conv slabs ----
        max_xpad_len = (MAX_SO + 2) * WP
        max_oslab_len = MAX_SO * WP
        max_stage = (MAX_SO + 2) * W
        for img in range(IMGS):
            h0 = 0
            for so in SLAB_PLAN[img]:
                slab_in = so + 2
                xpad_len = slab_in * WP
                xpad = xp.tile([P, max_xpad_len], f16, tag="xpad")
                xpad3 = xpad[:, :xpad_len].rearrange("p (r c) -> p r c", c=WP)
                # zero borders: cols {0,113} every row; pad row at image edge
                nc.gpsimd.memset(xpad3[:, :, 0:114:113], 0.0)
                if h0 == 0:
                    nc.gpsimd.memset(xpad[:, 0:WP], 0.0)
                elif h0 + so == H:
                    nc.gpsimd.memset(xpad[:, (slab_in - 1) * WP:xpad_len], 0.0)
                # interior rows: f32 staged load, scalar-engine cast to f16
                r_lo = max(0, h0 - 1)
                r_hi = min(H, h0 + so + 1)
                j0 = r_lo - (h0 - 1)
                nrows = r_hi - r_lo
                if img == 0 and h0 == 0:
                    stage = pre_stage
                else:
                    stage = xs.tile([P, max_stage], f32, tag="xstage")
                    nc.sync.dma_start(
                        stage[:, :nrows * W], x_t.ap()[img, :, r_lo:r_hi, :]
                    )
                nc.scalar.copy(
                    xpad3[:, j0:j0 + nrows, 1:1 + W],
                    stage[:, :nrows * W].rearrange("p (r c) -> p r c", c=W),
                )

                oslab = op.tile([P, max_oslab_len], f32, tag="oslab")
                q0 = 0
                for n in _slab_tiles(so):
                    pst = ps.tile([P, 512], f32, tag="pst")
                    for k in range(9):
                        dh, dw = divmod(k, 3)
                        off = q0 + dh * WP + dw
                        nc.tensor.matmul(
                            pst[:, :n],
                            w_k_view[:, k, :],
                            xpad[:, off:off + n],
                            start=(k == 0),
                            stop=(k == 8),
                        )
                    nc.vector.tensor_scalar_add(
                        oslab[:, q0:q0 + n], pst[:, :n], bias_s[:, 0:1]
                    )
                    q0 += n

                osrc = oslab[:, :so * WP].rearrange("p (r c) -> p r c", c=WP)[:, :, 0:W]
                nc.scalar.dma_start(out_t.ap()[img, :, h0:h0 + so, :], osrc)
                h0 += so

    nc.compile()
    return nc


def _make_in_maps(inputs):
    x = np.ascontiguousarray(np.asarray(inputs["x"], dtype=np.float32))
    cent = np.asarray(inputs["centroids"], dtype=np.float32).reshape(512, 9)
    idxT = np.asarray(inputs["idx"]).reshape(P, P).T          # [i, o]
    scalesT = np.ascontiguousarray(
        np.asarray(inputs["scales"], dtype=np.float32).reshape(P, P).T
    )
    cutT = np.ascontiguousarray(
        np.asarray(inputs["cut"], dtype=np.float32).reshape(P, P).T
    )
    bias = np.ascontiguousarray(
        np.asarray(inputs["bias"], dtype=np.float32).reshape(P, 1)
    )
    wrawT = np.ascontiguousarray(cent[idxT].reshape(P, P * 9))

    base = {"scalesT": scalesT, "cutT": cutT, "bias": bias, "wrawT": wrawT}
    maps = []
    for c in range(N_CORES):
        m = dict(base)
        m["x"] = np.ascontiguousarray(x[IMGS * c:IMGS * (c + 1)])
        maps.append(m)
    return maps


def _get_nc():
    if "nc" not in _CACHE:
        _CACHE["nc"] = _build()
    return _CACHE["nc"]


def _run(inputs, trace=False):
    nc = _get_nc()
    in_maps = _make_in_maps(inputs)
    res = bass_utils.run_bass_kernel_spmd(
        nc, in_maps, core_ids=list(range(N_CORES)), trace=trace
    )
    out = np.concatenate([res.results[c]["out"] for c in range(N_CORES)], axis=0)
    return out, res


def kernel(**inputs) -> np.ndarray:
    out, _ = _run(inputs, trace=False)
    return out

